# revision 51
# baseline (speedup 1.0000x reference)
"""ChildSum TreeLSTM on 8 trn2 NeuronCores (Bass/Tile, SPMD feature-split).

Strategy
--------
head[j] > j, so the tree is topologically ordered. Nodes are relabeled
level-contiguously (leaves first). Hidden dim H=1024 is feature-split
across 8 cores (128 features each). Per level (processed in batches of
<=512 nodes):

  gates_p = sigmoid/tanh(Wx_p + sum_{k in ch(p)} (U g h_k))

Linearity: z_k = U_cat h_k (U_cat = [U_i;U_o;U_u;U_f] row-blocked per
core) is needed per node. Each core computes the PARTIAL z (all 4096
outputs) from its own 128-feature h slice with 32 single-chunk fp16
matmuls, then a ReduceScatter(add, fp16) per batch group sums the
partials and hands each core its own 512-row slice. This replaces the
baseline's AllGather-of-h (4KB/node fp32) with 1KB/node fp16 on the
wire and kills the post-collective hT reload + full-contraction matmul.
Collectives are issued eagerly per batch (tiny trailing batches ride
the previous batch's collective) and their unpack is pipelined two
batches behind, so the collective queue stays saturated through the
large levels.

z slices are transposed to a node-major fp16 g_store; parents
segment-sum gathered g rows with a one-hot S matmul on the PE. The
forget path is nonlinear per child: fc_p = sum_k sigmoid(Wxf_p +
(U_f h_k)) * c_k, handled with gathered rows + elementwise + the same
S matmul. All matmuls run in fp16 (1.0 cycles/row at any size). Wx for
i/o/u stays resident in SBUF (24KB/partition) - no DRAM round trip.
"""
import numpy as np

N = 4096
H = 1024
HC = 128
NCORES = 8
PAD = N            # pad row index in node-major stores
BATCH = 512
CH = 128           # children per chunk
KCH = H // 128     # contraction chunks for U matmuls
KCHX = KCH + 1     # x contraction chunks incl. bias row
MAXNCH = 8
ZT = 32            # z out tiles (4096 / 128)


def _wrap_idx(a):
    """dma_gather index layout: idx[i] at [i%16, i//16], tiled to 128 partitions."""
    a = np.asarray(a, np.int64)
    n = len(a)
    c = (n + 15) // 16
    w = np.zeros((16, c), np.int16)
    w[np.arange(n) % 16, np.arange(n) // 16] = a.astype(np.int16)
    return np.tile(w, (8, 1))


def _schedule(head):
    head = np.asarray(head).astype(np.int64)
    n = head.shape[0]
    lev = np.zeros(n + 1, np.int64)
    for k in range(n):
        p = head[k]
        if lev[p] < lev[k] + 1:
            lev[p] = lev[k] + 1
    lv = lev[:n]
    order = np.argsort(lv, kind="stable")          # new -> old
    new_of_old = np.empty(n, np.int64)
    new_of_old[order] = np.arange(n)
    head_new = np.full(n, n, np.int64)
    for old in range(n):
        p = head[old]
        head_new[new_of_old[old]] = new_of_old[p] if p < n else n
    nlev = int(lv.max()) + 1
    mlev = [int((lv == L).sum()) for L in range(nlev)]
    start = np.concatenate([[0], np.cumsum(mlev)])
    kids = [[] for _ in range(n)]
    for k in range(n):
        p = head_new[k]
        if p < n:
            kids[p].append(k)

    batches = []
    for L in range(nlev):
        gs = int(start[L])
        while gs < start[L + 1]:
            bm = int(min(BATCH, start[L + 1] - gs))
            batches.append([L, gs, bm])
            gs += bm

    lv_new = np.empty(n, np.int64)
    for L in range(nlev):
        lv_new[start[L]:start[L + 1]] = L

    idx_blocks = []      # int16 wrapped blocks, concat on axis 1
    s_blocks = []        # [128, win] fp16 blocks
    icol = 0
    scol = 0
    binfos = []
    for (L, gs, bm) in batches:
        if L == 0:
            binfos.append(dict(L=L, gs=gs, bm=bm, chunks=[], nch=0,
                               s2col=None))
            continue
        # "fast" transition: the level-(L-1) children are summed straight
        # off the unpacked zred slab tiles (per-slab S2 matmuls, PSUM
        # accumulation), and only older children go through the gather.
        fast = L >= 2 and mlev[L - 1] <= 128 and bm <= 512
        chunks = []      # (wlo_rel, win, s_off_rel)
        slots_all = []
        wxf_all = []
        cur, curp = [], []
        plo = [None]
        phi = [None]

        def emit():
            padn = CH - len(cur)
            slots_all.extend(cur + [PAD] * padn)
            wxf_all.extend(curp + [PAD] * padn)
            if fast:
                wlo, win = 0, bm      # full-width for psum accumulation
            else:
                wlo, win = plo[0] - gs, phi[0] - plo[0] + 1
            S = np.zeros((CH, win), np.float16)
            for s in range(len(curp)):
                S[s, curp[s] - gs - wlo] = 1.0
            chunks.append((wlo, win))
            s_blocks.append(S)
            cur.clear()
            curp.clear()
            plo[0] = None

        for p in range(gs, gs + bm):
            ck = kids[p]
            assert 1 <= len(ck) <= CH
            if fast:
                ck = [k for k in ck if lv_new[k] != L - 1]
                if not ck:
                    continue
            if cur and len(cur) + len(ck) > CH:
                emit()
            if plo[0] is None:
                plo[0] = p
            phi[0] = p
            cur.extend(ck)
            curp.extend([p] * len(ck))
        if cur:
            emit()
        nch = len(chunks)
        assert nch <= MAXNCH, nch
        # per-chunk S col offsets (relative to this batch's scol)
        ch2 = []
        so = 0
        for (wlo, win) in chunks:
            ch2.append((wlo, win, so))
            so += win
        s2col = None
        nslp = 0
        if fast:
            nslp = (mlev[L - 1] + 127) // 128
            S2s = [np.zeros((CH, bm), np.float16) for _ in range(nslp)]
            for p in range(gs, gs + bm):
                for k in kids[p]:
                    if lv_new[k] == L - 1:
                        off = k - start[L - 1]
                        S2s[off // 128][off % 128, p - gs] = 1.0
            s_blocks.extend(S2s)
            s2col = scol + so
            so += bm * nslp
        binfo = dict(L=L, gs=gs, bm=bm, chunks=ch2, nch=nch,
                     scol=scol, scols=so, s2col=s2col, nslp=nslp)
        if nch:
            wi = _wrap_idx(slots_all)
            ww = _wrap_idx(wxf_all)
            binfo["icol_child"] = icol
            binfo["icol_wxf"] = icol + wi.shape[1]
            idx_blocks.append(wi)
            idx_blocks.append(ww)
            icol += wi.shape[1] + ww.shape[1]
        binfos.append(binfo)
        scol += so

    # per-level parent-of-node index blocks (for the fast-path Wxf gather)
    levels = [dict(gs=int(start[L]), bm=int(mlev[L])) for L in range(nlev)]
    for L in range(nlev):
        if mlev[L] <= 128:
            nsl = (mlev[L] + 127) // 128
            pidx = [int(head_new[k]) for k in range(start[L], start[L + 1])]
            pidx += [PAD] * (nsl * 128 - len(pidx))
            wpi = _wrap_idx(pidx)
            levels[L]["pwx_icol"] = icol
            idx_blocks.append(wpi)
            icol += wpi.shape[1]

    idxt = (np.concatenate(idx_blocks, axis=1) if idx_blocks
            else np.zeros((128, 1), np.int16))
    sall = (np.concatenate(s_blocks, axis=1) if s_blocks
            else np.zeros((128, 1), np.float16))
    return dict(order=order, new_of_old=new_of_old, nlev=nlev,
                batches=binfos, idxt=idxt, sall=sall, levels=levels)


def _build_nc(sched):
    import concourse.mybir as mybir
    import concourse.tile as tile
    from concourse import bacc
    from concourse.masks import make_identity

    F32 = mybir.dt.float32
    F16 = mybir.dt.float16
    I16 = mybir.dt.int16
    SIG = mybir.ActivationFunctionType.Sigmoid
    TANH = mybir.ActivationFunctionType.Tanh

    binfos = sched["batches"]
    nlev = sched["nlev"]
    levels = sched["levels"]
    icols = sched["idxt"].shape[1]
    scols = sched["sall"].shape[1]

    nc = bacc.Bacc("TRN2", target_bir_lowering=False, debug=False,
                   num_devices=NCORES)
    xT = nc.declare_dram_parameter("xT", [KCHX * 128, N], F16, isOutput=False)
    WT = nc.declare_dram_parameter("WT", [KCHX * 128, 512], F16, isOutput=False)
    UZT = nc.declare_dram_parameter("UZT", [128, ZT * 128], F16, isOutput=False)
    SALL = nc.declare_dram_parameter("SALL", [128, scols], F16, isOutput=False)
    IDXT = nc.declare_dram_parameter("IDXT", [128, icols], I16, isOutput=False)
    h_out = nc.declare_dram_parameter("h_out", [HC, N], F32, isOutput=True)
    c_out = nc.declare_dram_parameter("c_out", [N + 1, HC], F32, isOutput=True)

    g_store = nc.dram_tensor("g_store", [N + 1, 512], F16)
    wxf_store = nc.dram_tensor("wxf_store", [N + 1, HC], F16)

    # RS groups: one collective per BATCH. Single-batch groups allow a
    # partition-major zin/zout layout (one DMA descriptor per partition,
    # ~4x cheaper stores on the DMA-issue queue) and start the first
    # level-0 collective as soon as its batch's z is ready.
    lev_batches = {}
    for bi, b in enumerate(binfos):
        lev_batches.setdefault(b["L"], []).append(bi)
    groups = []                 # dict(gs0, bm, last_bi)
    grp_of_batch = {}
    lev_groups = {}             # level -> [group indices]
    for L in range(nlev - 1):
        bis = lev_batches[L]
        parts = [[bi] for bi in bis]
        # a tiny trailing batch rides the previous batch's collective
        # instead of paying its own 15us fixed cost
        if len(parts) >= 2 and binfos[bis[-1]]["bm"] < 64:
            parts = parts[:-2] + [parts[-2] + parts[-1]]
        for part in parts:
            gidx = len(groups)
            groups.append(dict(gs0=binfos[part[0]]["gs"],
                               bm=sum(binfos[i]["bm"] for i in part),
                               last_bi=part[-1], lev=L))
            lev_groups.setdefault(L, []).append(gidx)
            for i in part:
                grp_of_batch[i] = gidx
    zins = [nc.dram_tensor(f"zin{g}", [ZT * 128, grp["bm"]], F16)
            for g, grp in enumerate(groups)]
    zouts = [nc.dram_tensor(f"zout{g}", [512, grp["bm"]], F16)
             for g, grp in enumerate(groups)]

    ecnt = [0]
    pool_ok = [True]   # kept for emission bookkeeping; gpsimd cannot read
                       # PSUM on hw, so copies stay on vector/scalar

    def cpcopy(out, in_):
        ecnt[0] += 1
        if ecnt[0] % 2:
            nc.vector.tensor_copy(out, in_)
        else:
            nc.scalar.copy(out, in_)

    with tile.TileContext(nc) as tc:
        with (
            tc.tile_pool(name="const", bufs=1) as cpool,
            tc.tile_pool(name="xt", bufs=3) as xtp,
            tc.tile_pool(name="work", bufs=2) as wp,
            tc.tile_pool(name="gt", bufs=1) as gtp,
            tc.tile_pool(name="psA", bufs=1, space="PSUM") as psA,
            tc.tile_pool(name="pst", bufs=2, space="PSUM") as pst,
        ):
            ident = cpool.tile([128, 128], F32)
            make_identity(nc, ident[:])
            ident16 = cpool.tile([128, 128], F16)
            make_identity(nc, ident16[:])
            wt_sb = cpool.tile([128, KCHX, 512], F16)
            nc.sync.dma_start(wt_sb[:], WT[:].rearrange("(k p) j -> p k j", p=128))
            uz_sb = cpool.tile([128, ZT, 128], F16)
            nc.sync.dma_start(uz_sb[:], UZT[:].rearrange("p (t j) -> p t j", t=ZT))
            idx_sb = cpool.tile([128, icols], I16)
            nc.sync.dma_start(idx_sb[:], IDXT[:])
            sall_sb = cpool.tile([128, scols], F16)
            nc.sync.dma_start(sall_sb[:], SALL[:])
            zrow = cpool.tile([1, 512], F16)
            nc.vector.memset(zrow[:], 0.0)
            nc.sync.dma_start(g_store[N:N + 1, :], zrow[:, :])
            nc.sync.dma_start(wxf_store[N:N + 1, :], zrow[:, :HC])
            # Wx for i/o/u stays resident in SBUF, one tile per 512-node
            # chunk so level batches only depend on the chunks they read
            nchunks = N // 512
            wx_sb = [cpool.tile([128, 3, 512], F16, name=f"wx{ci}")
                     for ci in range(nchunks)]

            def emit_wx(ci):
                ps_wx = [psA.tile([128, 512], F32, tag=f"A{g}", name=f"pswx{g}")
                         for g in range(4)]
                for k in range(KCHX):
                    xt_t = xtp.tile([128, 512], F16, tag="xt")
                    nc.sync.dma_start(
                        xt_t[:], xT[k * 128:(k + 1) * 128, ci * 512:(ci + 1) * 512])
                    for g in range(4):
                        nc.tensor.matmul(
                            ps_wx[g][:], wt_sb[:, k, g * 128:(g + 1) * 128],
                            xt_t[:], start=(k == 0), stop=(k == KCHX - 1))
                for g in range(3):
                    cpcopy(wx_sb[ci][:, g, :], ps_wx[g][:])
                # f gate: transpose to node-major wxf_store
                tf = wp.tile([128, 512], F16, tag="wxf")
                cpcopy(tf[:], ps_wx[3][:])
                for s in range(4):
                    pt = pst.tile([128, 128], F16, tag="pt16")
                    nc.tensor.transpose(pt[:], tf[:, s * 128:(s + 1) * 128],
                                        ident16[:])
                    tnm = wp.tile([128, 128], F16, tag="wxfnm")
                    cpcopy(tnm[:], pt[:])
                    r0 = ci * 512 + s * 128
                    nc.sync.dma_start(wxf_store[r0:r0 + 128, :], tnm[:])

            def wx_add(out_ap, ps_ap, g, gs, bm):
                """out = ps + Wx_g[gs:gs+bm], reading piecewise from wx_sb."""
                pos, dst = gs, 0
                while pos < gs + bm:
                    ci2, off = pos // 512, pos % 512
                    take = min(512 - off, gs + bm - pos)
                    nc.vector.tensor_add(out_ap[:, dst:dst + take],
                                         ps_ap[:, dst:dst + take],
                                         wx_sb[ci2][:, g, off:off + take])
                    pos += take
                    dst += take

            def wx_act(out_ap, g, gs, bm, fn):
                pos, dst = gs, 0
                while pos < gs + bm:
                    ci2, off = pos // 512, pos % 512
                    take = min(512 - off, gs + bm - pos)
                    nc.scalar.activation(out_ap[:, dst:dst + take],
                                         wx_sb[ci2][:, g, off:off + take], fn)
                    pos += take
                    dst += take

            # ---------------- level phase ----------------
            def emit_batch(bi, mode="full", aux=None):
                b = binfos[bi]
                L = b["L"]
                gs, bm, nch = b["gs"], b["bm"], b["nch"]
                fast = b["s2col"] is not None
                ps_i = ps_o = ps_u = ps_f = None
                if L > 0 and (mode in ("full", "pre")):
                    ps_i = psA.tile([128, bm], F32, tag="A0")
                    ps_o = psA.tile([128, bm], F32, tag="A1")
                    ps_u = psA.tile([128, bm], F32, tag="A2")
                    ps_f = psA.tile([128, bm], F32, tag="A3")
                    if nch > 0:
                        co = b["icol_child"]
                        wo = b["icol_wxf"]
                        # one gather for all of z (i/o/u + f)
                        gi = gtp.tile([128, MAXNCH, 512], F16, tag="gi")
                        nc.gpsimd.dma_gather(
                            out_ap=gi[:, :nch, :], in_ap=g_store[:, :],
                            idxs_ap=idx_sb[:, co:co + nch * 8],
                            num_idxs=nch * 128, num_idxs_reg=nch * 128,
                            elem_size=512)
                        gc = gtp.tile([128, MAXNCH, 128], F32, tag="gc")
                        nc.gpsimd.dma_gather(
                            out_ap=gc[:, :nch, :], in_ap=c_out[:, :],
                            idxs_ap=idx_sb[:, co:co + nch * 8],
                            num_idxs=nch * 128, num_idxs_reg=nch * 128,
                            elem_size=128)
                        gw = gtp.tile([128, MAXNCH, 128], F16, tag="gw")
                        nc.gpsimd.dma_gather(
                            out_ap=gw[:, :nch, :], in_ap=wxf_store[:, :],
                            idxs_ap=idx_sb[:, wo:wo + nch * 8],
                            num_idxs=nch * 128, num_idxs_reg=nch * 128,
                            elem_size=128)
                        for cidx, (wlo, win, so) in enumerate(b["chunks"]):
                            sAP = sall_sb[:, b["scol"] + so: b["scol"] + so + win]
                            t1 = wp.tile([128, 128], F16, tag="fc1")
                            nc.vector.tensor_add(t1[:], gi[:, cidx, 384:512],
                                                 gw[:, cidx, :])
                            t2 = wp.tile([128, 128], F16, tag="fc2")
                            nc.scalar.activation(t2[:], t1[:], SIG)
                            t3 = wp.tile([128, 128], F16, tag="fc3")
                            nc.vector.tensor_mul(t3[:], t2[:], gc[:, cidx, :])
                            # fast batches accumulate full-width (S2 follows);
                            # normal batches hit disjoint windows
                            st = (cidx == 0) if fast else True
                            sp = (not fast) and True
                            nc.tensor.matmul(ps_f[:, wlo:wlo + win], t3[:], sAP,
                                             start=st, stop=sp)
                            nc.tensor.matmul(ps_i[:, wlo:wlo + win],
                                             gi[:, cidx, 0:128], sAP,
                                             start=st, stop=sp)
                            nc.tensor.matmul(ps_o[:, wlo:wlo + win],
                                             gi[:, cidx, 128:256], sAP,
                                             start=st, stop=sp)
                            nc.tensor.matmul(ps_u[:, wlo:wlo + win],
                                             gi[:, cidx, 256:384], sAP,
                                             start=st, stop=sp)
                if mode == "pre":
                    return (ps_i, ps_o, ps_u, ps_f)
                if mode == "post":
                    ps_i, ps_o, ps_u, ps_f = aux["ps"]
                    slabs, gwr, gcr = aux["slabs"], aux["gwr"], aux["gcr"]
                    nslp = len(slabs)
                    for s, (gnm_s, sw_s) in enumerate(slabs):
                        s2ap = sall_sb[:, b["s2col"] + s * bm:
                                       b["s2col"] + (s + 1) * bm]
                        tr1 = wp.tile([128, 128], F16, tag="fc1")
                        nc.vector.tensor_add(tr1[:], gnm_s[:, 384:512],
                                             gwr[:, s, :])
                        tr2 = wp.tile([128, 128], F16, tag="fc2")
                        nc.scalar.activation(tr2[:], tr1[:], SIG)
                        tr3 = wp.tile([128, 128], F16, tag="fc3")
                        nc.vector.tensor_mul(tr3[:], tr2[:], gcr[:, s, :])
                        st = (nch == 0 and s == 0)
                        sp = (s == nslp - 1)
                        nc.tensor.matmul(ps_f[:], tr3[:], s2ap,
                                         start=st, stop=sp)
                        nc.tensor.matmul(ps_i[:], gnm_s[:, 0:128], s2ap,
                                         start=st, stop=sp)
                        nc.tensor.matmul(ps_o[:], gnm_s[:, 128:256], s2ap,
                                         start=st, stop=sp)
                        nc.tensor.matmul(ps_u[:], gnm_s[:, 256:384], s2ap,
                                         start=st, stop=sp)
                i_sb = wp.tile([128, bm], F32, tag="isb")
                    o_sb = wp.tile([128, bm], F32, tag="osb")
                    u_sb = wp.tile([128, bm], F32, tag="usb")
                    c_sb = wp.tile([128, bm], F32, tag="csb")
                    h_sb = wp.tile([128, bm], F32, tag="hsb")
                    if L == 0:
                        wx_act(i_sb, 0, gs, bm, SIG)
                        wx_act(o_sb, 1, gs, bm, SIG)
                        wx_act(u_sb, 2, gs, bm, TANH)
                        nc.vector.tensor_mul(c_sb[:], i_sb[:], u_sb[:])
                    else:
                        t = wp.tile([128, bm], F32, tag="gtmp1")
                        wx_add(t, ps_i, 0, gs, bm)
                        nc.scalar.activation(i_sb[:], t[:], SIG)
                        t = wp.tile([128, bm], F32, tag="gtmp2")
                        wx_add(t, ps_o, 1, gs, bm)
                        nc.scalar.activation(o_sb[:], t[:], SIG)
                        t = wp.tile([128, bm], F32, tag="gtmp3")
                        wx_add(t, ps_u, 2, gs, bm)
                        nc.scalar.activation(u_sb[:], t[:], TANH)
                        t = wp.tile([128, bm], F32, tag="gtmp4")
                        nc.vector.tensor_mul(t[:], i_sb[:], u_sb[:])
                        nc.vector.tensor_add(c_sb[:], t[:], ps_f[:])
                    th = wp.tile([128, bm], F32, tag="thsb")
                    nc.scalar.activation(th[:], c_sb[:], TANH)
                    nc.vector.tensor_mul(h_sb[:], o_sb[:], th[:])
                    if L < nlev - 1:
                        # z partials: 32 fp16 matmuls from own h slice.
                        # Pack tpb out-tiles per PSUM bank so small batches
                        # need few psum->sbuf copies and one DMA.
                        h16 = wp.tile([128, bm], F16, tag="h16")
                        nc.scalar.copy(h16[:], h_sb[:])
                        gidx = grp_of_batch[bi]
                        c0 = gs - groups[gidx]["gs0"]
                        tpb = 1
                        while tpb < 8 and tpb * 2 * bm <= 512:
                            tpb *= 2
                        wsz = 4 * tpb          # tiles per wave (4 psum tags)
                        for w0 in range(0, ZT, wsz):
                            zsb = wp.tile([128, wsz, bm], F16, tag="zsb",
                                          bufs=3)
                            for t4 in range(4):
                                psz = psA.tile([128, tpb, bm], F32,
                                               tag=f"A{t4}")
                                for ti in range(tpb):
                                    t_ = w0 + t4 * tpb + ti
                                    nc.tensor.matmul(
                                        psz[:, ti, :], uz_sb[:, t_, :], h16[:],
                                        start=True, stop=True)
                                cpcopy(zsb[:, t4 * tpb:(t4 + 1) * tpb, :],
                                       psz[:])
                            nc.sync.dma_start(
                                zins[gidx][w0 * 128:(w0 + wsz) * 128,
                                           c0:c0 + bm]
                                .rearrange("(t p) j -> p t j", p=128),
                                zsb[:])
                    nc.sync.dma_start(h_out[:, gs:gs + bm], h_sb[:])
                    for s in range((bm + 127) // 128):
                        sw = min(128, bm - s * 128)
                        pt = pst.tile([128, 128], F32, tag="pt")
                        nc.tensor.transpose(
                            pt[:sw, :], c_sb[:, s * 128:s * 128 + sw], ident[:])
                        tnm = wp.tile([128, 128], F32, tag="cnm")
                        cpcopy(tnm[:sw, :], pt[:sw, :])
                        r0 = gs + s * 128
                        nc.sync.dma_start(c_out[r0:r0 + sw, :], tnm[:sw, :])

            def emit_rs(gidx):
                nc.gpsimd.collective_compute(
                    "ReduceScatter", mybir.AluOpType.add,
                    replica_groups=[list(range(NCORES))],
                    ins=[zins[gidx][:, :]], outs=[zouts[gidx][:, :]])

            lev_slabs = {}   # level -> [(gnm tile, sw)] in slab order

            def emit_unpack(gidx):
                # Transpose the zred slices node-major into g_store
                # (slab-pipelined); register each slab tile so a fast
                # successor level can consume it straight from SBUF.
                grp = groups[gidx]
                gs0, bmG = grp["gs0"], grp["bm"]
                for s in range((bmG + 127) // 128):
                    sw = min(128, bmG - s * 128)
                    r0 = gs0 + s * 128
                    zr = gtp.tile([128, 4, 128], F16, tag="zr", bufs=3)
                    nc.sync.dma_start(
                        zr[:, :, :sw],
                        zouts[gidx][:, s * 128:s * 128 + sw]
                        .rearrange("(g p) j -> p g j", p=128))
                    gnm = wp.tile([128, 512], F16, tag="gnm", bufs=2)
                    if sw < 128:
                        nc.vector.memset(gnm[:], 0.0)
                    for g in range(4):
                        pt = pst.tile([128, 128], F16, tag="pt16")
                        nc.tensor.transpose(
                            pt[:sw, :], zr[:, g, :sw], ident16[:])
                        cpcopy(gnm[:sw, g * 128:(g + 1) * 128], pt[:sw, :])
                    nc.sync.dma_start(g_store[r0:r0 + sw, :], gnm[:sw, :])
                    lev_slabs.setdefault(grp["lev"], []).append((gnm, sw))

            # Emission order: interleave Wx chunks with the level-0 batches
            # that consume them (a level-0 batch needs only its own chunk).
            # Collectives are issued as soon as their group's z data is
            # queued, but the post-RS unpack (which WAITS on the collective)
            # is deferred so it doesn't head-of-line-block the in-order DMA
            # queue for later batches' stores.
            emitted_wx = 0
            l0_groups = []
            for bi in lev_batches[0]:
                b = binfos[bi]
                need = (b["gs"] + b["bm"] - 1) // 512 + 1
                while emitted_wx < need:
                    emit_wx(emitted_wx)
                    emitted_wx += 1
                emit_batch(bi)
                gidx = grp_of_batch.get(bi)
                if gidx is not None and groups[gidx]["last_bi"] == bi:
                    emit_rs(gidx)
                    l0_groups.append(gidx)
                    pool_ok[0] = False
                    # unpack two batches behind the RS pipeline: its
                    # collective is long done, so no queue stall
                    if len(l0_groups) >= 3:
                        emit_unpack(l0_groups[-3])
            while emitted_wx < nchunks:
                emit_wx(emitted_wx)
                emitted_wx += 1
            for gidx in l0_groups[-2:]:
                emit_unpack(gidx)
            pool_ok[0] = True
            # Levels 1+: level (L-1)'s collective is issued here, AFTER
            # level L's fast-path prep (gathers of old children, Wxf/c of
            # the fast path) so that prep overlaps the collective.
            for L in range(1, nlev):
                bis = lev_batches[L]
                prevfast = binfos[bis[0]]["s2col"] is not None
                aux = None
                if L >= 2:
                    pres = {}
                    if prevfast:
                        pL = levels[L - 1]
                        pw = pL["pwx_icol"]
                        nslp = (pL["bm"] + 127) // 128
                        gwr = gtp.tile([128, 1, 128], F16, tag="gwr")
                        nc.gpsimd.dma_gather(
                            out_ap=gwr[:, :, :], in_ap=wxf_store[:, :],
                            idxs_ap=idx_sb[:, pw:pw + 8],
                            num_idxs=128, num_idxs_reg=128,
                            elem_size=128)
                        gcr = gtp.tile([128, 1, 128], F32, tag="gcr")
                        nc.vector.memset(gcr[:], 0.0)
                        nc.sync.dma_start(
                            gcr[:pL["bm"], 0, :],
                            c_out[pL["gs"]:pL["gs"] + pL["bm"], :])
                        for bi in bis:
                            pres[bi] = emit_batch(bi, mode="pre")
                    # previous level's FINAL collective issued here (earlier
                    # groups were issued AND unpacked within that level).
                    pgs = lev_groups[L - 1]
                    emit_rs(pgs[-1])
                    emit_unpack(pgs[-1])
                    if prevfast:
                        aux = dict(slabs=lev_slabs[L - 1], gwr=gwr, gcr=gcr)
                pool_ok[0] = True
                for bi in bis:
                    if prevfast and L >= 2:
                        a = dict(aux)
                        a["ps"] = pres[bi]
                        emit_batch(bi, mode="post", aux=a)
                    else:
                        emit_batch(bi)
                    gidx = grp_of_batch.get(bi)
                    if (gidx is not None and L < nlev - 1
                            and gidx != lev_groups[L][-1]):
                        emit_rs(gidx)   # eager RS for non-final batches
                        pool_ok[0] = False
                # non-final groups' unpacks run inside this level; the final
                # one follows its RS at the next level's iteration
                if L < nlev - 1:
                    for g_ in lev_groups[L][:-1]:
                        emit_unpack(g_)

    nc.finalize()
    return nc


def _in_maps(x, kw, sched):
    order = sched["order"]
    n = x.shape[0]
    # xT padded with bias row at row H (ones), zeros after; columns in new order
    xT = np.zeros((KCHX * 128, n), np.float16)
    xT[:H, :] = x[order].T.astype(np.float16)
    xT[H, :] = 1.0

    Ws = {g: np.asarray(kw[f"W_{g}"], np.float32) for g in "iouf"}
    Us = {g: np.asarray(kw[f"U_{g}"], np.float32) for g in "iouf"}
    bs = {g: np.asarray(kw[f"b_{g}"], np.float32) for g in "iouf"}

    in_maps = []
    for d in range(NCORES):
        dsl = slice(d * HC, (d + 1) * HC)
        WT = np.zeros((KCHX * 128, 512), np.float16)
        UZT = np.zeros((128, ZT * 128), np.float16)
        for gi_, g in enumerate("iouf"):
            WT[:H, gi_ * 128:(gi_ + 1) * 128] = Ws[g][dsl, :].T.astype(np.float16)
            WT[H, gi_ * 128:(gi_ + 1) * 128] = bs[g][dsl].astype(np.float16)
        for c in range(NCORES):
            for gi_, g in enumerate("iouf"):
                t = c * 4 + gi_
                # lhsT tile for out rows [c*512+g*128 : +128], contraction =
                # this core d's h slice: (U_g[c_slice, d_slice])^T
                UZT[:, t * 128:(t + 1) * 128] = \
                    Us[g][c * HC:(c + 1) * HC, dsl].T.astype(np.float16)
        in_maps.append({
            "xT": xT, "WT": WT, "UZT": UZT,
            "SALL": np.ascontiguousarray(sched["sall"]),
            "IDXT": np.ascontiguousarray(sched["idxt"]),
        })
    return in_maps


def kernel(x=None, head=None, **kw):
    import concourse.mybir as mybir  # noqa: F401  (env check)
    from concourse.bass_utils import run_bass_kernel_spmd

    x = np.asarray(x, np.float32)
    head_np = np.asarray(head)
    sched = _schedule(head_np)
    new_of_old = sched["new_of_old"]
    n = x.shape[0]
    in_maps = _in_maps(x, kw, sched)

    nc = _build_nc(sched)
    res = run_bass_kernel_spmd(nc, in_maps, list(range(NCORES)))

    h_new = np.concatenate([res.results[c]["h_out"] for c in range(NCORES)], axis=0).T
    c_new = np.concatenate([res.results[c]["c_out"][:n] for c in range(NCORES)], axis=1)
    h = h_new[new_of_old]
    cc = c_new[new_of_old]
    return h, cc


# revision 57
# speedup vs baseline: 1.0110x; 1.0110x over previous
"""ChildSum TreeLSTM on 8 trn2 NeuronCores (Bass/Tile, SPMD feature-split).

Strategy
--------
head[j] > j, so the tree is topologically ordered. Nodes are relabeled
level-contiguously (leaves first). Hidden dim H=1024 is feature-split
across 8 cores (128 features each). Per level (processed in batches of
<=512 nodes):

  gates_p = sigmoid/tanh(Wx_p + sum_{k in ch(p)} (U g h_k))

Linearity: z_k = U_cat h_k (U_cat = [U_i;U_o;U_u;U_f] row-blocked per
core) is needed per node. Each core computes the PARTIAL z (all 4096
outputs) from its own 128-feature h slice with 32 single-chunk fp16
matmuls, then a ReduceScatter(add, fp16) per batch group sums the
partials and hands each core its own 512-row slice. This replaces the
baseline's AllGather-of-h (4KB/node fp32) with 1KB/node fp16 on the
wire and kills the post-collective hT reload + full-contraction matmul.
Collectives are issued eagerly per batch (tiny trailing batches ride
the previous batch's collective) and their unpack is pipelined two
batches behind, so the collective queue stays saturated through the
large levels.

z slices are transposed to a node-major fp16 g_store; parents
segment-sum gathered g rows with a one-hot S matmul on the PE. The
forget path is nonlinear per child: fc_p = sum_k sigmoid(Wxf_p +
(U_f h_k)) * c_k, handled with gathered rows + elementwise + the same
S matmul. All matmuls run in fp16 (1.0 cycles/row at any size). Wx for
i/o/u stays resident in SBUF (24KB/partition) - no DRAM round trip.
"""
import numpy as np

N = 4096
H = 1024
HC = 128
NCORES = 8
PAD = N            # pad row index in node-major stores
BATCH = 512
CH = 128           # children per chunk
KCH = H // 128     # contraction chunks for U matmuls
KCHX = KCH + 1     # x contraction chunks incl. bias row
MAXNCH = 8
ZT = 32            # z out tiles (4096 / 128)


def _wrap_idx(a):
    """dma_gather index layout: idx[i] at [i%16, i//16], tiled to 128 partitions."""
    a = np.asarray(a, np.int64)
    n = len(a)
    c = (n + 15) // 16
    w = np.zeros((16, c), np.int16)
    w[np.arange(n) % 16, np.arange(n) // 16] = a.astype(np.int16)
    return np.tile(w, (8, 1))


def _schedule(head):
    head = np.asarray(head).astype(np.int64)
    n = head.shape[0]
    lev = np.zeros(n + 1, np.int64)
    for k in range(n):
        p = head[k]
        if lev[p] < lev[k] + 1:
            lev[p] = lev[k] + 1
    lv = lev[:n]
    order = np.argsort(lv, kind="stable")          # new -> old
    new_of_old = np.empty(n, np.int64)
    new_of_old[order] = np.arange(n)
    head_new = np.full(n, n, np.int64)
    for old in range(n):
        p = head[old]
        head_new[new_of_old[old]] = new_of_old[p] if p < n else n
    nlev = int(lv.max()) + 1
    mlev = [int((lv == L).sum()) for L in range(nlev)]
    start = np.concatenate([[0], np.cumsum(mlev)])
    kids = [[] for _ in range(n)]
    for k in range(n):
        p = head_new[k]
        if p < n:
            kids[p].append(k)

    batches = []
    for L in range(nlev):
        gs = int(start[L])
        while gs < start[L + 1]:
            bm = int(min(BATCH, start[L + 1] - gs))
            batches.append([L, gs, bm])
            gs += bm

    lv_new = np.empty(n, np.int64)
    for L in range(nlev):
        lv_new[start[L]:start[L + 1]] = L

    idx_blocks = []      # int16 wrapped blocks, concat on axis 1
    s_blocks = []        # [128, win] fp16 blocks
    icol = 0
    scol = 0
    binfos = []
    for (L, gs, bm) in batches:
        if L == 0:
            binfos.append(dict(L=L, gs=gs, bm=bm, chunks=[], nch=0,
                               s2col=None))
            continue
        # "fast" transition: the level-(L-1) children are summed straight
        # off the unpacked zred slab tiles (per-slab S2 matmuls, PSUM
        # accumulation), and only older children go through the gather.
        fast = L >= 2 and mlev[L - 1] <= 128 and bm <= 512
        chunks = []      # (wlo_rel, win, s_off_rel)
        slots_all = []
        wxf_all = []
        cur, curp = [], []
        plo = [None]
        phi = [None]

        def emit():
            padn = CH - len(cur)
            slots_all.extend(cur + [PAD] * padn)
            wxf_all.extend(curp + [PAD] * padn)
            if fast:
                wlo, win = 0, bm      # full-width for psum accumulation
            else:
                wlo, win = plo[0] - gs, phi[0] - plo[0] + 1
            S = np.zeros((CH, win), np.float16)
            for s in range(len(curp)):
                S[s, curp[s] - gs - wlo] = 1.0
            chunks.append((wlo, win))
            s_blocks.append(S)
            cur.clear()
            curp.clear()
            plo[0] = None

        for p in range(gs, gs + bm):
            ck = kids[p]
            assert 1 <= len(ck) <= CH
            if fast:
                ck = [k for k in ck if lv_new[k] != L - 1]
                if not ck:
                    continue
            if cur and len(cur) + len(ck) > CH:
                emit()
            if plo[0] is None:
                plo[0] = p
            phi[0] = p
            cur.extend(ck)
            curp.extend([p] * len(ck))
        if cur:
            emit()
        nch = len(chunks)
        assert nch <= MAXNCH, nch
        # per-chunk S col offsets (relative to this batch's scol)
        ch2 = []
        so = 0
        for (wlo, win) in chunks:
            ch2.append((wlo, win, so))
            so += win
        s2col = None
        nslp = 0
        if fast:
            nslp = (mlev[L - 1] + 127) // 128
            S2s = [np.zeros((CH, bm), np.float16) for _ in range(nslp)]
            for p in range(gs, gs + bm):
                for k in kids[p]:
                    if lv_new[k] == L - 1:
                        off = k - start[L - 1]
                        S2s[off // 128][off % 128, p - gs] = 1.0
            s_blocks.extend(S2s)
            s2col = scol + so
            so += bm * nslp
        binfo = dict(L=L, gs=gs, bm=bm, chunks=ch2, nch=nch,
                     scol=scol, scols=so, s2col=s2col, nslp=nslp)
        if nch:
            wi = _wrap_idx(slots_all)
            ww = _wrap_idx(wxf_all)
            binfo["icol_child"] = icol
            binfo["icol_wxf"] = icol + wi.shape[1]
            idx_blocks.append(wi)
            idx_blocks.append(ww)
            icol += wi.shape[1] + ww.shape[1]
        binfos.append(binfo)
        scol += so

    # per-level parent-of-node index blocks (for the fast-path Wxf gather)
    levels = [dict(gs=int(start[L]), bm=int(mlev[L])) for L in range(nlev)]
    for L in range(nlev):
        if mlev[L] <= 128:
            nsl = (mlev[L] + 127) // 128
            pidx = [int(head_new[k]) for k in range(start[L], start[L + 1])]
            pidx += [PAD] * (nsl * 128 - len(pidx))
            wpi = _wrap_idx(pidx)
            levels[L]["pwx_icol"] = icol
            idx_blocks.append(wpi)
            icol += wpi.shape[1]

    idxt = (np.concatenate(idx_blocks, axis=1) if idx_blocks
            else np.zeros((128, 1), np.int16))
    sall = (np.concatenate(s_blocks, axis=1) if s_blocks
            else np.zeros((128, 1), np.float16))
    return dict(order=order, new_of_old=new_of_old, nlev=nlev,
                batches=binfos, idxt=idxt, sall=sall, levels=levels)


def _build_nc(sched):
    import concourse.mybir as mybir
    import concourse.tile as tile
    from concourse import bacc
    from concourse.masks import make_identity

    F32 = mybir.dt.float32
    F16 = mybir.dt.float16
    I16 = mybir.dt.int16
    SIG = mybir.ActivationFunctionType.Sigmoid
    TANH = mybir.ActivationFunctionType.Tanh

    binfos = sched["batches"]
    nlev = sched["nlev"]
    levels = sched["levels"]
    icols = sched["idxt"].shape[1]
    scols = sched["sall"].shape[1]

    nc = bacc.Bacc("TRN2", target_bir_lowering=False, debug=False,
                   num_devices=NCORES)
    xT = nc.declare_dram_parameter("xT", [KCHX * 128, N], F16, isOutput=False)
    WT = nc.declare_dram_parameter("WT", [KCHX * 128, 512], F16, isOutput=False)
    UZT = nc.declare_dram_parameter("UZT", [128, ZT * 128], F16, isOutput=False)
    SALL = nc.declare_dram_parameter("SALL", [128, scols], F16, isOutput=False)
    IDXT = nc.declare_dram_parameter("IDXT", [128, icols], I16, isOutput=False)
    h_out = nc.declare_dram_parameter("h_out", [HC, N], F32, isOutput=True)
    c_out = nc.declare_dram_parameter("c_out", [N + 1, HC], F32, isOutput=True)

    g_store = nc.dram_tensor("g_store", [N + 1, 512], F16)
    wxf_store = nc.dram_tensor("wxf_store", [N + 1, HC], F16)

    # RS groups: one collective per BATCH. Single-batch groups allow a
    # partition-major zin/zout layout (one DMA descriptor per partition,
    # ~4x cheaper stores on the DMA-issue queue) and start the first
    # level-0 collective as soon as its batch's z is ready.
    lev_batches = {}
    for bi, b in enumerate(binfos):
        lev_batches.setdefault(b["L"], []).append(bi)
    groups = []                 # dict(gs0, bm, last_bi)
    grp_of_batch = {}
    lev_groups = {}             # level -> [group indices]
    for L in range(nlev - 1):
        bis = lev_batches[L]
        parts = [[bi] for bi in bis]
        # a tiny trailing batch rides the previous batch's collective
        # instead of paying its own 15us fixed cost
        if len(parts) >= 2 and binfos[bis[-1]]["bm"] < 64:
            parts = parts[:-2] + [parts[-2] + parts[-1]]
        for part in parts:
            gidx = len(groups)
            groups.append(dict(gs0=binfos[part[0]]["gs"],
                               bm=sum(binfos[i]["bm"] for i in part),
                               last_bi=part[-1], lev=L))
            lev_groups.setdefault(L, []).append(gidx)
            for i in part:
                grp_of_batch[i] = gidx
    zins = [nc.dram_tensor(f"zin{g}", [ZT * 128, grp["bm"]], F16)
            for g, grp in enumerate(groups)]
    zouts = [nc.dram_tensor(f"zout{g}", [512, grp["bm"]], F16)
             for g, grp in enumerate(groups)]

    ecnt = [0]
    pool_ok = [True]   # kept for emission bookkeeping; gpsimd cannot read
                       # PSUM on hw, so copies stay on vector/scalar

    def cpcopy(out, in_):
        ecnt[0] += 1
        if ecnt[0] % 2:
            nc.vector.tensor_copy(out, in_)
        else:
            nc.scalar.copy(out, in_)

    with tile.TileContext(nc) as tc:
        with (
            tc.tile_pool(name="const", bufs=1) as cpool,
            tc.tile_pool(name="xt", bufs=3) as xtp,
            tc.tile_pool(name="work", bufs=2) as wp,
            tc.tile_pool(name="gt", bufs=1) as gtp,
            tc.tile_pool(name="psA", bufs=1, space="PSUM") as psA,
            tc.tile_pool(name="pst", bufs=2, space="PSUM") as pst,
        ):
            ident = cpool.tile([128, 128], F32)
            make_identity(nc, ident[:])
            ident16 = cpool.tile([128, 128], F16)
            make_identity(nc, ident16[:])
            wt_sb = cpool.tile([128, KCHX, 512], F16)
            nc.sync.dma_start(wt_sb[:], WT[:].rearrange("(k p) j -> p k j", p=128))
            uz_sb = cpool.tile([128, ZT, 128], F16)
            idx_sb = cpool.tile([128, icols], I16)
            sall_sb = cpool.tile([128, scols], F16)
            zrow = cpool.tile([1, 512], F16)

            def emit_late_consts():
                # issued after the first Wx chunk's loads so they don't
                # delay the critical head on the DMA-issue queue; none are
                # consumed before the first z matmuls / level-1 gathers
                nc.sync.dma_start(
                    uz_sb[:], UZT[:].rearrange("p (t j) -> p t j", t=ZT))
                nc.sync.dma_start(idx_sb[:], IDXT[:])
                nc.sync.dma_start(sall_sb[:], SALL[:])
                nc.vector.memset(zrow[:], 0.0)
                nc.sync.dma_start(g_store[N:N + 1, :], zrow[:, :])
                nc.sync.dma_start(wxf_store[N:N + 1, :], zrow[:, :HC])
            # Wx for i/o/u stays resident in SBUF, one tile per 512-node
            # chunk so level batches only depend on the chunks they read
            nchunks = N // 512
            wx_sb = [cpool.tile([128, 3, 512], F16, name=f"wx{ci}")
                     for ci in range(nchunks)]

            def emit_wx(ci):
                ps_wx = [psA.tile([128, 512], F32, tag=f"A{g}", name=f"pswx{g}")
                         for g in range(4)]
                for k in range(KCHX):
                    xt_t = xtp.tile([128, 512], F16, tag="xt")
                    nc.sync.dma_start(
                        xt_t[:], xT[k * 128:(k + 1) * 128, ci * 512:(ci + 1) * 512])
                    for g in range(4):
                        nc.tensor.matmul(
                            ps_wx[g][:], wt_sb[:, k, g * 128:(g + 1) * 128],
                            xt_t[:], start=(k == 0), stop=(k == KCHX - 1))
                for g in range(3):
                    cpcopy(wx_sb[ci][:, g, :], ps_wx[g][:])
                # f gate: transpose to node-major wxf_store
                tf = wp.tile([128, 512], F16, tag="wxf")
                cpcopy(tf[:], ps_wx[3][:])
                for s in range(4):
                    pt = pst.tile([128, 128], F16, tag="pt16")
                    nc.tensor.transpose(pt[:], tf[:, s * 128:(s + 1) * 128],
                                        ident16[:])
                    tnm = wp.tile([128, 128], F16, tag="wxfnm")
                    cpcopy(tnm[:], pt[:])
                    r0 = ci * 512 + s * 128
                    nc.sync.dma_start(wxf_store[r0:r0 + 128, :], tnm[:])

            def wx_add(out_ap, ps_ap, g, gs, bm):
                """out = ps + Wx_g[gs:gs+bm], reading piecewise from wx_sb."""
                pos, dst = gs, 0
                while pos < gs + bm:
                    ci2, off = pos // 512, pos % 512
                    take = min(512 - off, gs + bm - pos)
                    nc.vector.tensor_add(out_ap[:, dst:dst + take],
                                         ps_ap[:, dst:dst + take],
                                         wx_sb[ci2][:, g, off:off + take])
                    pos += take
                    dst += take

            def wx_act(out_ap, g, gs, bm, fn):
                pos, dst = gs, 0
                while pos < gs + bm:
                    ci2, off = pos // 512, pos % 512
                    take = min(512 - off, gs + bm - pos)
                    nc.scalar.activation(out_ap[:, dst:dst + take],
                                         wx_sb[ci2][:, g, off:off + take], fn)
                    pos += take
                    dst += take

            # ---------------- level phase ----------------
            def emit_batch(bi, mode="full", aux=None):
                b = binfos[bi]
                L = b["L"]
                gs, bm, nch = b["gs"], b["bm"], b["nch"]
                fast = b["s2col"] is not None
                ps_i = ps_o = ps_u = ps_f = None
                if L > 0 and (mode in ("full", "pre")):
                    ps_i = psA.tile([128, bm], F32, tag="A0")
                    ps_o = psA.tile([128, bm], F32, tag="A1")
                    ps_u = psA.tile([128, bm], F32, tag="A2")
                    ps_f = psA.tile([128, bm], F32, tag="A3")
                    if nch > 0:
                        co = b["icol_child"]
                        wo = b["icol_wxf"]
                        # one gather for all of z (i/o/u + f)
                        gi = gtp.tile([128, MAXNCH, 512], F16, tag="gi")
                        nc.gpsimd.dma_gather(
                            out_ap=gi[:, :nch, :], in_ap=g_store[:, :],
                            idxs_ap=idx_sb[:, co:co + nch * 8],
                            num_idxs=nch * 128, num_idxs_reg=nch * 128,
                            elem_size=512)
                        gc = gtp.tile([128, MAXNCH, 128], F32, tag="gc")
                        nc.gpsimd.dma_gather(
                            out_ap=gc[:, :nch, :], in_ap=c_out[:, :],
                            idxs_ap=idx_sb[:, co:co + nch * 8],
                            num_idxs=nch * 128, num_idxs_reg=nch * 128,
                            elem_size=128)
                        gw = gtp.tile([128, MAXNCH, 128], F16, tag="gw")
                        nc.gpsimd.dma_gather(
                            out_ap=gw[:, :nch, :], in_ap=wxf_store[:, :],
                            idxs_ap=idx_sb[:, wo:wo + nch * 8],
                            num_idxs=nch * 128, num_idxs_reg=nch * 128,
                            elem_size=128)
                        vadd = nc.gpsimd if L >= 2 else nc.vector
                        for cidx, (wlo, win, so) in enumerate(b["chunks"]):
                            sAP = sall_sb[:, b["scol"] + so: b["scol"] + so + win]
                            t1 = wp.tile([128, 128], F16, tag="fc1")
                            vadd.tensor_add(t1[:], gi[:, cidx, 384:512],
                                            gw[:, cidx, :])
                            t2 = wp.tile([128, 128], F16, tag="fc2")
                            nc.scalar.activation(t2[:], t1[:], SIG)
                            t3 = wp.tile([128, 128], F16, tag="fc3")
                            vadd.tensor_mul(t3[:], t2[:], gc[:, cidx, :])
                            # fast batches accumulate full-width (S2 follows);
                            # normal batches hit disjoint windows
                            st = (cidx == 0) if fast else True
                            sp = (not fast) and True
                            nc.tensor.matmul(ps_f[:, wlo:wlo + win], t3[:], sAP,
                                             start=st, stop=sp)
                            nc.tensor.matmul(ps_i[:, wlo:wlo + win],
                                             gi[:, cidx, 0:128], sAP,
                                             start=st, stop=sp)
                            nc.tensor.matmul(ps_o[:, wlo:wlo + win],
                                             gi[:, cidx, 128:256], sAP,
                                             start=st, stop=sp)
                            nc.tensor.matmul(ps_u[:, wlo:wlo + win],
                                             gi[:, cidx, 256:384], sAP,
                                             start=st, stop=sp)
                if mode == "pre":
                    return (ps_i, ps_o, ps_u, ps_f)
                if mode == "post":
                    ps_i, ps_o, ps_u, ps_f = aux["ps"]
                    slabs, gwr, gcr = aux["slabs"], aux["gwr"], aux["gcr"]
                    nslp = len(slabs)
                    for s, (gnm_s, sw_s) in enumerate(slabs):
                        s2ap = sall_sb[:, b["s2col"] + s * bm:
                                       b["s2col"] + (s + 1) * bm]
                        tr1 = wp.tile([128, 128], F16, tag="fc1")
                        nc.gpsimd.tensor_add(tr1[:], gnm_s[:, 384:512],
                                             gwr[:, s, :])
                        tr2 = wp.tile([128, 128], F16, tag="fc2")
                        nc.scalar.activation(tr2[:], tr1[:], SIG)
                        tr3 = wp.tile([128, 128], F16, tag="fc3")
                        nc.gpsimd.tensor_mul(tr3[:], tr2[:], gcr[:, s, :])
                        st = (nch == 0 and s == 0)
                        sp = (s == nslp - 1)
                        nc.tensor.matmul(ps_f[:], tr3[:], s2ap,
                                         start=st, stop=sp)
                        nc.tensor.matmul(ps_i[:], gnm_s[:, 0:128], s2ap,
                                         start=st, stop=sp)
                        nc.tensor.matmul(ps_o[:], gnm_s[:, 128:256], s2ap,
                                         start=st, stop=sp)
                        nc.tensor.matmul(ps_u[:], gnm_s[:, 256:384], s2ap,
                                         start=st, stop=sp)
                i_sb = wp.tile([128, bm], F32, tag="isb")
                    o_sb = wp.tile([128, bm], F32, tag="osb")
                    u_sb = wp.tile([128, bm], F32, tag="usb")
                    c_sb = wp.tile([128, bm], F32, tag="csb")
                    h_sb = wp.tile([128, bm], F32, tag="hsb")
                    if L == 0:
                        wx_act(i_sb, 0, gs, bm, SIG)
                        wx_act(o_sb, 1, gs, bm, SIG)
                        wx_act(u_sb, 2, gs, bm, TANH)
                        nc.vector.tensor_mul(c_sb[:], i_sb[:], u_sb[:])
                    else:
                        t = wp.tile([128, bm], F32, tag="gtmp1")
                        wx_add(t, ps_i, 0, gs, bm)
                        nc.scalar.activation(i_sb[:], t[:], SIG)
                        t = wp.tile([128, bm], F32, tag="gtmp2")
                        wx_add(t, ps_o, 1, gs, bm)
                        nc.scalar.activation(o_sb[:], t[:], SIG)
                        t = wp.tile([128, bm], F32, tag="gtmp3")
                        wx_add(t, ps_u, 2, gs, bm)
                        nc.scalar.activation(u_sb[:], t[:], TANH)
                        t = wp.tile([128, bm], F32, tag="gtmp4")
                        nc.vector.tensor_mul(t[:], i_sb[:], u_sb[:])
                        nc.vector.tensor_add(c_sb[:], t[:], ps_f[:])
                    th = wp.tile([128, bm], F32, tag="thsb")
                    nc.scalar.activation(th[:], c_sb[:], TANH)
                    nc.vector.tensor_mul(h_sb[:], o_sb[:], th[:])
                    if L < nlev - 1:
                        # z partials: 32 fp16 matmuls from own h slice.
                        # Pack tpb out-tiles per PSUM bank so small batches
                        # need few psum->sbuf copies and one DMA.
                        h16 = wp.tile([128, bm], F16, tag="h16")
                        nc.scalar.copy(h16[:], h_sb[:])
                        gidx = grp_of_batch[bi]
                        c0 = gs - groups[gidx]["gs0"]
                        tpb = 1
                        while tpb < 8 and tpb * 2 * bm <= 512:
                            tpb *= 2
                        wsz = 4 * tpb          # tiles per wave (4 psum tags)
                        for w0 in range(0, ZT, wsz):
                            zsb = wp.tile([128, wsz, bm], F16, tag="zsb",
                                          bufs=3)
                            for t4 in range(4):
                                psz = psA.tile([128, tpb, bm], F32,
                                               tag=f"A{t4}")
                                for ti in range(tpb):
                                    t_ = w0 + t4 * tpb + ti
                                    nc.tensor.matmul(
                                        psz[:, ti, :], uz_sb[:, t_, :], h16[:],
                                        start=True, stop=True)
                                cpcopy(zsb[:, t4 * tpb:(t4 + 1) * tpb, :],
                                       psz[:])
                            nc.sync.dma_start(
                                zins[gidx][w0 * 128:(w0 + wsz) * 128,
                                           c0:c0 + bm]
                                .rearrange("(t p) j -> p t j", p=128),
                                zsb[:])
                    nc.sync.dma_start(h_out[:, gs:gs + bm], h_sb[:])
                    for s in range((bm + 127) // 128):
                        sw = min(128, bm - s * 128)
                        pt = pst.tile([128, 128], F32, tag="pt")
                        nc.tensor.transpose(
                            pt[:sw, :], c_sb[:, s * 128:s * 128 + sw], ident[:])
                        tnm = wp.tile([128, 128], F32, tag="cnm")
                        cpcopy(tnm[:sw, :], pt[:sw, :])
                        r0 = gs + s * 128
                        nc.sync.dma_start(c_out[r0:r0 + sw, :], tnm[:sw, :])

            def emit_rs(gidx):
                nc.gpsimd.collective_compute(
                    "ReduceScatter", mybir.AluOpType.add,
                    replica_groups=[list(range(NCORES))],
                    ins=[zins[gidx][:, :]], outs=[zouts[gidx][:, :]])

            lev_slabs = {}   # level -> [(gnm tile, sw)] in slab order

            def emit_unpack(gidx):
                # Transpose the zred slices node-major into g_store
                # (slab-pipelined); register each slab tile so a fast
                # successor level can consume it straight from SBUF.
                grp = groups[gidx]
                gs0, bmG = grp["gs0"], grp["bm"]
                for s in range((bmG + 127) // 128):
                    sw = min(128, bmG - s * 128)
                    r0 = gs0 + s * 128
                    zr = gtp.tile([128, 4, 128], F16, tag="zr", bufs=3)
                    nc.sync.dma_start(
                        zr[:, :, :sw],
                        zouts[gidx][:, s * 128:s * 128 + sw]
                        .rearrange("(g p) j -> p g j", p=128))
                    gnm = wp.tile([128, 512], F16, tag="gnm", bufs=2)
                    if sw < 128:
                        nc.vector.memset(gnm[:], 0.0)
                    for g in range(4):
                        pt = pst.tile([128, 128], F16, tag="pt16")
                        nc.tensor.transpose(
                            pt[:sw, :], zr[:, g, :sw], ident16[:])
                        cpcopy(gnm[:sw, g * 128:(g + 1) * 128], pt[:sw, :])
                    nc.sync.dma_start(g_store[r0:r0 + sw, :], gnm[:sw, :])
                    lev_slabs.setdefault(grp["lev"], []).append((gnm, sw))

            # Emission order: interleave Wx chunks with the level-0 batches
            # that consume them (a level-0 batch needs only its own chunk).
            # Collectives are issued as soon as their group's z data is
            # queued, but the post-RS unpack (which WAITS on the collective)
            # is deferred so it doesn't head-of-line-block the in-order DMA
            # queue for later batches' stores.
            emitted_wx = 0
            l0_groups = []
            for bi in lev_batches[0]:
                b = binfos[bi]
                need = (b["gs"] + b["bm"] - 1) // 512 + 1
                while emitted_wx < need:
                    emit_wx(emitted_wx)
                    emitted_wx += 1
                    if emitted_wx == 1:
                        emit_late_consts()
                emit_batch(bi)
                gidx = grp_of_batch.get(bi)
                if gidx is not None and groups[gidx]["last_bi"] == bi:
                    emit_rs(gidx)
                    l0_groups.append(gidx)
                    pool_ok[0] = False
                    # unpack two batches behind the RS pipeline: its
                    # collective is long done, so no queue stall
                    if len(l0_groups) >= 3:
                        emit_unpack(l0_groups[-3])
            while emitted_wx < nchunks:
                emit_wx(emitted_wx)
                emitted_wx += 1
            for gidx in l0_groups[-2:]:
                emit_unpack(gidx)
            pool_ok[0] = True
            # Levels 1+: level (L-1)'s collective is issued here, AFTER
            # level L's fast-path prep (gathers of old children, Wxf/c of
            # the fast path) so that prep overlaps the collective.
            for L in range(1, nlev):
                bis = lev_batches[L]
                prevfast = binfos[bis[0]]["s2col"] is not None
                aux = None
                if L >= 2:
                    pres = {}
                    if prevfast:
                        pL = levels[L - 1]
                        pw = pL["pwx_icol"]
                        nslp = (pL["bm"] + 127) // 128
                        gwr = gtp.tile([128, 1, 128], F16, tag="gwr")
                        nc.gpsimd.dma_gather(
                            out_ap=gwr[:, :, :], in_ap=wxf_store[:, :],
                            idxs_ap=idx_sb[:, pw:pw + 8],
                            num_idxs=128, num_idxs_reg=128,
                            elem_size=128)
                        gcr = gtp.tile([128, 1, 128], F32, tag="gcr")
                        nc.vector.memset(gcr[:], 0.0)
                        nc.sync.dma_start(
                            gcr[:pL["bm"], 0, :],
                            c_out[pL["gs"]:pL["gs"] + pL["bm"], :])
                        for bi in bis:
                            pres[bi] = emit_batch(bi, mode="pre")
                    # previous level's FINAL collective issued here (earlier
                    # groups were issued AND unpacked within that level).
                    pgs = lev_groups[L - 1]
                    emit_rs(pgs[-1])
                    emit_unpack(pgs[-1])
                    if prevfast:
                        aux = dict(slabs=lev_slabs[L - 1], gwr=gwr, gcr=gcr)
                pool_ok[0] = True
                if prevfast and L >= 2:
                    for bi in bis:
                        a = dict(aux)
                        a["ps"] = pres[bi]
                        emit_batch(bi, mode="post", aux=a)
                elif len(bis) > 1:
                    # multi-batch level: issue later batches' gathers (and
                    # their S-matmuls) BEFORE the first eager collective, so
                    # the in-order Pool queue doesn't head-of-line block them
                    # behind it
                    emit_batch(bis[0])
                    pres2 = {bi: emit_batch(bi, mode="pre")
                             for bi in bis[1:]}
                    for k, bi in enumerate(bis):
                        if k > 0:
                            emit_batch(bi, mode="post",
                                       aux=dict(ps=pres2[bi], slabs=[],
                                                gwr=None, gcr=None))
                        gidx = grp_of_batch.get(bi)
                        if (gidx is not None and L < nlev - 1
                                and gidx != lev_groups[L][-1]):
                            emit_rs(gidx)   # eager RS for non-final batches
                else:
                    emit_batch(bis[0])
                # non-final groups' unpacks run inside this level; the final
                # one follows its RS at the next level's iteration
                if L < nlev - 1:
                    for g_ in lev_groups[L][:-1]:
                        emit_unpack(g_)

    nc.finalize()
    return nc


def _in_maps(x, kw, sched):
    order = sched["order"]
    n = x.shape[0]
    # xT padded with bias row at row H (ones), zeros after; columns in new order
    xT = np.zeros((KCHX * 128, n), np.float16)
    xT[:H, :] = x[order].T.astype(np.float16)
    xT[H, :] = 1.0

    Ws = {g: np.asarray(kw[f"W_{g}"], np.float32) for g in "iouf"}
    Us = {g: np.asarray(kw[f"U_{g}"], np.float32) for g in "iouf"}
    bs = {g: np.asarray(kw[f"b_{g}"], np.float32) for g in "iouf"}

    in_maps = []
    for d in range(NCORES):
        dsl = slice(d * HC, (d + 1) * HC)
        WT = np.zeros((KCHX * 128, 512), np.float16)
        UZT = np.zeros((128, ZT * 128), np.float16)
        for gi_, g in enumerate("iouf"):
            WT[:H, gi_ * 128:(gi_ + 1) * 128] = Ws[g][dsl, :].T.astype(np.float16)
            WT[H, gi_ * 128:(gi_ + 1) * 128] = bs[g][dsl].astype(np.float16)
        for c in range(NCORES):
            for gi_, g in enumerate("iouf"):
                t = c * 4 + gi_
                # lhsT tile for out rows [c*512+g*128 : +128], contraction =
                # this core d's h slice: (U_g[c_slice, d_slice])^T
                UZT[:, t * 128:(t + 1) * 128] = \
                    Us[g][c * HC:(c + 1) * HC, dsl].T.astype(np.float16)
        in_maps.append({
            "xT": xT, "WT": WT, "UZT": UZT,
            "SALL": np.ascontiguousarray(sched["sall"]),
            "IDXT": np.ascontiguousarray(sched["idxt"]),
        })
    return in_maps


def kernel(x=None, head=None, **kw):
    import concourse.mybir as mybir  # noqa: F401  (env check)
    from concourse.bass_utils import run_bass_kernel_spmd

    x = np.asarray(x, np.float32)
    head_np = np.asarray(head)
    sched = _schedule(head_np)
    new_of_old = sched["new_of_old"]
    n = x.shape[0]
    in_maps = _in_maps(x, kw, sched)

    nc = _build_nc(sched)
    res = run_bass_kernel_spmd(nc, in_maps, list(range(NCORES)))

    h_new = np.concatenate([res.results[c]["h_out"] for c in range(NCORES)], axis=0).T
    c_new = np.concatenate([res.results[c]["c_out"][:n] for c in range(NCORES)], axis=1)
    h = h_new[new_of_old]
    cc = c_new[new_of_old]
    return h, cc
